# revision 8
# baseline (speedup 1.0000x reference)
"""Multi-headed attention kernel for Trainium2, SPMD across 8 NeuronCores.

Problem: B=4, S=2048, D_MODEL=1024, HEADS=16, D_HEAD=64 (fp32).

Sharding (per hint): batch across cores (4 batches x 2 cores each); within a
batch pair, heads are split 8+8 (tensor parallel). Each core computes, for its
(batch b, head half hh):
    Q^T = (Wq_s)^T X_q^T + bq   [512, 2048]   (hd-major layout, f32)
    K^T = (Wk_s)^T X_k^T + bk   [512, 2048]
    V'' = X_v Wv_s + bv         [2048, 8*65]  (bf16; per-head 65th col = 8.0)
    per head h, sq-half:  L^T = K_h Q_h^T  (f32r matmuls, PSUM f32)
                          P = exp(L^T) (bf16)  *  maskT01 (bf16, elementwise)
                          AV^T[65,sq] += V''_h^T P   (bf16 matmuls)
                          attn^T = AV^T[0:64] * (1 / AV^T[64])   (row 64 = 8*rowsum)
    out^T = Wo_s^T attn^T       [1024, 2048]  (partial over heads)
Host: out[b] = (outT_core0 + outT_core1).T + bo.

The mask is applied multiplicatively after exp: exp(l + (-1e8)) == exp(l)*0
exactly in fp32 (underflow to 0), identical to the reference's where().
No row-max subtraction is needed: logits are O(+-50), exp stays finite in f32.
"""
import numpy as np
import ml_dtypes
from contextlib import ExitStack

import concourse.bass as bass
import concourse.tile as tile
from concourse import bacc, mybir
from concourse.bass_utils import run_bass_kernel_spmd

F32 = mybir.dt.float32
F32R = mybir.dt.float32r
BF16 = mybir.dt.bfloat16

B, S, D, H, DH = 4, 2048, 1024, 16, 64
HPC = 8           # heads per core
HD = HPC * DH     # 512 head-dims per core
NCORES = 8
ET = D // 128     # 8 e-tiles (d_model contraction tiles)
ST = S // 128     # 16 s-tiles
ADD = mybir.AluOpType.add

# stash of last run results for test harness introspection
last_results = None


def _emit(ctx: ExitStack, tc: tile.TileContext, io: dict):
    nc = tc.nc
    xqT, xkT, xvT = io["xqT"], io["xkT"], io["xvT"]
    wq, wk, wv, wo = io["wq"], io["wk"], io["wv"], io["wo"]
    bqh, bkh, bvh = io["bqh"], io["bkh"], io["bvh"]
    maskT, outT = io["maskT"], io["outT"]

    const = ctx.enter_context(tc.tile_pool(name="const", bufs=1))
    bigA = ctx.enter_context(tc.tile_pool(name="bigA", bufs=1))

    # ---- constants ----
    bqh_sb = const.tile([128, 4], F32)
    nc.sync.dma_start(bqh_sb, bqh)
    bkh_sb = const.tile([128, 4], F32)
    nc.sync.dma_start(bkh_sb, bkh)
    bv_bc = const.tile([128, HD], F32)
    nc.gpsimd.dma_start(bv_bc, bvh.partition_broadcast(128))

    # ---- tensors spanning phases A..C ----
    qT = bigA.tile([128, 4, S], F32R)       # [p, hd-tile, sq]
    kT = bigA.tile([128, 4, S], F32R)
    vs = bigA.tile([128, ST, HPC * 65], BF16)  # [p, s-tile, h*65+d]; col 64 per head = 8.0

    # ===== Phase A: projections =====
    with tc.tile_pool(name="xin", bufs=9) as xin, \
         tc.tile_pool(name="wpool", bufs=2) as wpool, \
         tc.tile_pool(name="pa", bufs=4, space="PSUM") as pa:
        # --- Q^T and K^T (orientation: out[hd, sq] = W^T @ X^T) ---
        for which, (xT, w_dram, bias_sb, out_sb) in enumerate(
            [(xqT, wq, bqh_sb, qT), (xkT, wk, bkh_sb, kT)]
        ):
            w_sb = wpool.tile([128, ET, HD], F32R, tag="w", name=f"w{which}")
            nc.sync.dma_start(w_sb, w_dram.rearrange("(e p) f -> p e f", p=128))
            xts = []
            for e in range(ET):
                xt = xin.tile([128, S], F32R, tag="x", name=f"x{which}_{e}")
                nc.sync.dma_start(xt, xT[e * 128:(e + 1) * 128, :])
                xts.append(xt)
            for i in range(4):
                for c in range(4):
                    ps = pa.tile([128, 512], F32, name="ps_qk")
                    for e in range(ET):
                        nc.tensor.matmul(
                            ps,
                            w_sb[:, e, i * 128:(i + 1) * 128],
                            xts[e][:, c * 512:(c + 1) * 512],
                            start=(e == 0), stop=(e == ET - 1),
                        )
                    nc.vector.tensor_scalar(
                        out=out_sb[:, i, c * 512:(c + 1) * 512], in0=ps,
                        scalar1=bias_sb[:, i:i + 1], scalar2=None, op0=ADD,
                    )

        # --- V'' (orientation: out[s, hd] = X @ Wv) ---
        wv_sb = wpool.tile([128, ET, HD], F32R, tag="w")
        nc.sync.dma_start(wv_sb, wv.rearrange("(e p) f -> p e f", p=128))
        xvs = []
        for e in range(ET):
            xt = xin.tile([128, S], F32R, tag="x", name=f"xv_{e}")
            nc.sync.dma_start(xt, xvT[e * 128:(e + 1) * 128, :])
            xvs.append(xt)
        ones_view = vs.rearrange("p s (h dd) -> p s h dd", dd=65)[:, :, :, 64:65]
        nc.vector.memset(ones_view, 8.0)
        for s in range(ST):
            ps = pa.tile([128, 512], F32, name="ps_v")
            for e in range(ET):
                nc.tensor.matmul(
                    ps,
                    xvs[e][:, s * 128:(s + 1) * 128],
                    wv_sb[:, e, :],
                    start=(e == 0), stop=(e == ET - 1),
                )
            nc.vector.tensor_add(
                out=vs[:, s, :].rearrange("p (h dd) -> p h dd", dd=65)[:, :, 0:64],
                in0=ps.rearrange("p (h d) -> p h d", d=64),
                in1=bv_bc.rearrange("p (h d) -> p h d", d=64),
            )

    with tc.tile_pool(name="attp", bufs=1) as attp:
        att = attp.tile([128, 4, S], F32R)   # attn^T [hd, sq]

        # ===== Phase B: attention =====
        with tc.tile_pool(name="mskp", bufs=1) as mskp, \
             tc.tile_pool(name="ppool", bufs=2) as ppool, \
             tc.tile_pool(name="rpool", bufs=2) as rpool, \
             tc.tile_pool(name="rdram", bufs=2, space="DRAM") as rdram, \
             tc.tile_pool(name="pqk", bufs=2, space="PSUM") as pqk, \
             tc.tile_pool(name="pav", bufs=2, space="PSUM") as pav:
            msk = mskp.tile([128, ST, S], BF16)   # maskT01 [sk, sq]
            for t in range(ST):
                nc.sync.dma_start(msk[:, t, :], maskT[t * 128:(t + 1) * 128, :])

            for g in range(4):            # head pairs; bases 0/64 overlap on PE
                for Hh in range(2):       # sq halves
                    avs = []
                    for hl in range(2):
                        av = pav.tile([65, 1024], F32, tag="av", name=f"av{hl}")
                        avs.append(av)
                    for t in range(ST):
                        pts = []
                        for hl in range(2):
                            h = 2 * g + hl
                            r0 = hl * 64
                            qk = pqk.tile([128, 1024], F32, name="qk")
                            for c2 in range(2):
                                nc.tensor.matmul(
                                    qk[:, c2 * 512:(c2 + 1) * 512],
                                    kT[r0:r0 + 64, g, t * 128:(t + 1) * 128],
                                    qT[r0:r0 + 64, g,
                                       Hh * 1024 + c2 * 512:Hh * 1024 + (c2 + 1) * 512],
                                    start=True, stop=True,
                                )
                            p = ppool.tile([128, 1024], BF16, name="p")
                            nc.scalar.activation(
                                p, qk, mybir.ActivationFunctionType.Exp)
                            nc.vector.tensor_mul(
                                p, p, msk[:, t, Hh * 1024:(Hh + 1) * 1024])
                            pts.append((h, p))
                        for hl, (h, p) in enumerate(pts):
                            for c2 in range(2):
                                nc.tensor.matmul(
                                    avs[hl][:, c2 * 512:(c2 + 1) * 512],
                                    vs[:, t, h * 65:(h + 1) * 65],
                                    p[:, c2 * 512:(c2 + 1) * 512],
                                    start=(t == 0), stop=(t == ST - 1),
                                )
                    for hl in range(2):
                        h = 2 * g + hl
                        r0 = hl * 64
                        av = avs[hl]
                        rs = rpool.tile([65, 1024], F32, tag="rs", name="rs")
                        nc.vector.reciprocal(rs[64:65, :], av[64:65, :])
                        rd = rdram.tile([1, 1024], F32, tag="rd", name="rd")
                        nc.sync.dma_start(rd, rs[64:65, :])
                        rbc = rpool.tile([64, 1024], F32, tag="rbc", name="rbc")
                        nc.gpsimd.dma_start(
                            rbc, rd.partition_broadcast(64))
                        nc.vector.tensor_mul(
                            att[r0:r0 + 64, g, Hh * 1024:(Hh + 1) * 1024],
                            av[0:64, :], rbc)

        # ===== Phase C: output projection =====
        with tc.tile_pool(name="wpc", bufs=1) as wpc, \
             tc.tile_pool(name="ostg", bufs=2) as ostg, \
             tc.tile_pool(name="po", bufs=4, space="PSUM") as po:
            wo_sb = wpc.tile([128, 4, D], F32R)
            nc.sync.dma_start(wo_sb, wo.rearrange("(j p) f -> p j f", p=128))
            for dm in range(ET):
                stg = ostg.tile([128, S], F32, name="stg")
                for c in range(4):
                    ps = po.tile([128, 512], F32, name="ps_o")
                    for j in range(4):
                        nc.tensor.matmul(
                            ps,
                            wo_sb[:, j, dm * 128:(dm + 1) * 128],
                            att[:, j, c * 512:(c + 1) * 512],
                            start=(j == 0), stop=(j == 3),
                        )
                    nc.scalar.copy(stg[:, c * 512:(c + 1) * 512], ps)
                nc.sync.dma_start(outT[dm * 128:(dm + 1) * 128, :], stg)


def build_nc():
    nc = bacc.Bacc("TRN2", target_bir_lowering=False, debug=False,
                   num_devices=NCORES)
    io = {}
    for name, shape, dt_, kind in [
        ("xqT", [D, S], F32R, "ExternalInput"),
        ("xkT", [D, S], F32R, "ExternalInput"),
        ("xvT", [D, S], F32R, "ExternalInput"),
        ("wq", [D, HD], F32R, "ExternalInput"),
        ("wk", [D, HD], F32R, "ExternalInput"),
        ("wv", [D, HD], F32R, "ExternalInput"),
        ("wo", [HD, D], F32R, "ExternalInput"),
        ("bqh", [128, 4], F32, "ExternalInput"),
        ("bkh", [128, 4], F32, "ExternalInput"),
        ("bvh", [HD], F32, "ExternalInput"),
        ("maskT", [S, S], BF16, "ExternalInput"),
        ("outT", [D, S], F32, "ExternalOutput"),
    ]:
        io[name] = nc.dram_tensor(name, shape, dt_, kind=kind).ap()
    with tile.TileContext(nc) as tc:
        with ExitStack() as ctx:
            _emit(ctx, tc, io)
    nc.compile()
    return nc


def make_in_maps(query, key_, value, mask, Wq, bq, Wk, bk, Wv, bv, Wo, bo):
    in_maps = []
    for c in range(NCORES):
        b, hh = c // 2, c % 2
        h0 = hh * HPC
        m01T = np.ascontiguousarray((~mask[b]).T).astype(ml_dtypes.bfloat16)
        in_maps.append({
            "xqT": np.ascontiguousarray(query[b].T),
            "xkT": np.ascontiguousarray(key_[b].T),
            "xvT": np.ascontiguousarray(value[b].T),
            "wq": np.ascontiguousarray(Wq[:, h0:h0 + HPC, :].reshape(D, HD)),
            "wk": np.ascontiguousarray(Wk[:, h0:h0 + HPC, :].reshape(D, HD)),
            "wv": np.ascontiguousarray(Wv[:, h0:h0 + HPC, :].reshape(D, HD)),
            "wo": np.ascontiguousarray(Wo[h0:h0 + HPC].reshape(HD, D)),
            "bqh": np.ascontiguousarray(
                bq[h0:h0 + HPC].reshape(4, 128).T),
            "bkh": np.ascontiguousarray(
                bk[h0:h0 + HPC].reshape(4, 128).T),
            "bvh": np.ascontiguousarray(bv[h0:h0 + HPC].reshape(HD)),
            "maskT": m01T,
        })
    return in_maps


_nc_cache = None


def kernel(query, key_, value, mask, Wq, bq, Wk, bk, Wv, bv, Wo, bo):
    global last_results, _nc_cache
    query = np.asarray(query, dtype=np.float32)
    key_ = np.asarray(key_, dtype=np.float32)
    value = np.asarray(value, dtype=np.float32)
    mask = np.asarray(mask, dtype=bool)
    Wq, bq = np.asarray(Wq, np.float32), np.asarray(bq, np.float32)
    Wk, bk = np.asarray(Wk, np.float32), np.asarray(bk, np.float32)
    Wv, bv = np.asarray(Wv, np.float32), np.asarray(bv, np.float32)
    Wo, bo = np.asarray(Wo, np.float32), np.asarray(bo, np.float32)

    if _nc_cache is None:
        _nc_cache = build_nc()
    in_maps = make_in_maps(query, key_, value, mask, Wq, bq, Wk, bk,
                           Wv, bv, Wo, bo)
    res = run_bass_kernel_spmd(_nc_cache, in_maps, core_ids=list(range(NCORES)))
    last_results = res
    out = np.empty((B, S, D), dtype=np.float32)
    for b in range(B):
        acc = res.results[2 * b]["outT"].astype(np.float32) + \
            res.results[2 * b + 1]["outT"].astype(np.float32)
        out[b] = acc.T + bo[None, :]
    return out


# revision 9
# speedup vs baseline: 1.0560x; 1.0560x over previous
"""Multi-headed attention kernel for Trainium2, SPMD across 8 NeuronCores.

Problem: B=4, S=2048, D_MODEL=1024, HEADS=16, D_HEAD=64 (fp32).

Sharding (per hint): batch across cores (4 batches x 2 cores each); within a
batch pair, heads are split 8+8 (tensor parallel). Each core computes, for its
(batch b, head half hh):
    Q^T = (Wq_s)^T X_q^T + bq   [512, 2048]   (hd-major layout, f32)
    K^T = (Wk_s)^T X_k^T + bk   [512, 2048]
    V'' = X_v Wv_s + bv         [2048, 8*65]  (bf16; per-head 65th col = 8.0)
    per head h, sq-half:  L^T = K_h Q_h^T  (f32r matmuls, PSUM f32)
                          P = exp(L^T) (bf16)  *  maskT01 (bf16, elementwise)
                          AV^T[65,sq] += V''_h^T P   (bf16 matmuls)
                          attn^T = AV^T[0:64] * (1 / AV^T[64])   (row 64 = 8*rowsum)
    out^T = Wo_s^T attn^T       [1024, 2048]  (partial over heads)
Host: out[b] = (outT_core0 + outT_core1).T + bo.

The mask is applied multiplicatively after exp: exp(l + (-1e8)) == exp(l)*0
exactly in fp32 (underflow to 0), identical to the reference's where().
No row-max subtraction is needed: logits are O(+-50), exp stays finite in f32.
"""
import numpy as np
import ml_dtypes
from contextlib import ExitStack

import concourse.bass as bass
import concourse.tile as tile
from concourse import bacc, mybir
from concourse.bass_utils import run_bass_kernel_spmd

F32 = mybir.dt.float32
F32R = mybir.dt.float32r
BF16 = mybir.dt.bfloat16

B, S, D, H, DH = 4, 2048, 1024, 16, 64
HPC = 8           # heads per core
HD = HPC * DH     # 512 head-dims per core
NCORES = 8
ET = D // 128     # 8 e-tiles (d_model contraction tiles)
ST = S // 128     # 16 s-tiles
ADD = mybir.AluOpType.add

# stash of last run results for test harness introspection
last_results = None


def _emit(ctx: ExitStack, tc: tile.TileContext, io: dict):
    nc = tc.nc
    xqT, xkT, xvT = io["xqT"], io["xkT"], io["xvT"]
    wq, wk, wv, wo = io["wq"], io["wk"], io["wv"], io["wo"]
    bqh, bkh, bvh = io["bqh"], io["bkh"], io["bvh"]
    maskT, outT = io["maskT"], io["outT"]

    const = ctx.enter_context(tc.tile_pool(name="const", bufs=1))
    bigA = ctx.enter_context(tc.tile_pool(name="bigA", bufs=1))

    # ---- constants ----
    bqh_sb = const.tile([128, 4], F32)
    nc.sync.dma_start(bqh_sb, bqh)
    bkh_sb = const.tile([128, 4], F32)
    nc.sync.dma_start(bkh_sb, bkh)
    bv_bc = const.tile([128, HD], F32)
    nc.gpsimd.dma_start(bv_bc, bvh.partition_broadcast(128))

    # ---- tensors spanning phases A..C ----
    qT = bigA.tile([128, 4, S], F32R)       # [p, hd-tile, sq]
    kT = bigA.tile([128, 4, S], F32R)
    vs = bigA.tile([128, ST, HPC * 65], BF16)  # [p, s-tile, h*65+d]; col 64 per head = 8.0

    # ===== Phase A: projections =====
    with tc.tile_pool(name="xin", bufs=9) as xin, \
         tc.tile_pool(name="wpool", bufs=2) as wpool, \
         tc.tile_pool(name="pa", bufs=4, space="PSUM") as pa:
        # --- Q^T and K^T (orientation: out[hd, sq] = W^T @ X^T) ---
        for which, (xT, w_dram, bias_sb, out_sb) in enumerate(
            [(xqT, wq, bqh_sb, qT), (xkT, wk, bkh_sb, kT)]
        ):
            w_sb = wpool.tile([128, ET, HD], F32R, tag="w", name=f"w{which}")
            nc.sync.dma_start(w_sb, w_dram.rearrange("(e p) f -> p e f", p=128))
            xts = []
            for e in range(ET):
                xt = xin.tile([128, S], F32R, tag="x", name=f"x{which}_{e}")
                nc.sync.dma_start(xt, xT[e * 128:(e + 1) * 128, :])
                xts.append(xt)
            for i in range(4):
                for c in range(4):
                    ps = pa.tile([128, 512], F32, name="ps_qk")
                    for e in range(ET):
                        nc.tensor.matmul(
                            ps,
                            w_sb[:, e, i * 128:(i + 1) * 128],
                            xts[e][:, c * 512:(c + 1) * 512],
                            start=(e == 0), stop=(e == ET - 1),
                        )
                    nc.vector.tensor_scalar(
                        out=out_sb[:, i, c * 512:(c + 1) * 512], in0=ps,
                        scalar1=bias_sb[:, i:i + 1], scalar2=None, op0=ADD,
                    )

        # --- V'' (orientation: out[s, hd] = X @ Wv) ---
        wv_sb = wpool.tile([128, ET, HD], F32R, tag="w")
        nc.sync.dma_start(wv_sb, wv.rearrange("(e p) f -> p e f", p=128))
        xvs = []
        for e in range(ET):
            xt = xin.tile([128, S], F32R, tag="x", name=f"xv_{e}")
            nc.sync.dma_start(xt, xvT[e * 128:(e + 1) * 128, :])
            xvs.append(xt)
        ones_view = vs.rearrange("p s (h dd) -> p s h dd", dd=65)[:, :, :, 64:65]
        nc.vector.memset(ones_view, 8.0)
        for s in range(ST):
            ps = pa.tile([128, 512], F32, name="ps_v")
            for e in range(ET):
                nc.tensor.matmul(
                    ps,
                    xvs[e][:, s * 128:(s + 1) * 128],
                    wv_sb[:, e, :],
                    start=(e == 0), stop=(e == ET - 1),
                )
            nc.vector.tensor_add(
                out=vs[:, s, :].rearrange("p (h dd) -> p h dd", dd=65)[:, :, 0:64],
                in0=ps.rearrange("p (h d) -> p h d", d=64),
                in1=bv_bc.rearrange("p (h d) -> p h d", d=64),
            )

    with tc.tile_pool(name="attp", bufs=1) as attp:
        att = attp.tile([128, 4, S], F32R)   # attn^T [hd, sq]

        # ===== Phase B: attention =====
        with tc.tile_pool(name="mskp", bufs=1) as mskp, \
             tc.tile_pool(name="ppool", bufs=2) as ppool, \
             tc.tile_pool(name="rpool", bufs=2) as rpool, \
             tc.tile_pool(name="rdram", bufs=2, space="DRAM") as rdram, \
             tc.tile_pool(name="pqk", bufs=2, space="PSUM") as pqk, \
             tc.tile_pool(name="pav", bufs=2, space="PSUM") as pav:
            msk = mskp.tile([128, ST, S], BF16)   # maskT01 [sk, sq]
            for t in range(ST):
                nc.sync.dma_start(msk[:, t, :], maskT[t * 128:(t + 1) * 128, :])

            for g in range(4):            # head pairs; bases 0/64 overlap on PE
                for Hh in range(2):       # sq halves
                    avs = []
                    for hl in range(2):
                        av = pav.tile([65, 1024], F32, tag="av", name=f"av{hl}")
                        avs.append(av)
                    for t in range(ST):
                        # QK: interleave the two heads' matmuls so the
                        # K=64 row strips (bases 0/64) stream concurrently.
                        qks = []
                        for hl in range(2):
                            qks.append(pqk.tile([128, 1024], F32, name="qk"))
                        for c2 in range(2):
                            for hl in range(2):
                                r0 = hl * 64
                                nc.tensor.matmul(
                                    qks[hl][:, c2 * 512:(c2 + 1) * 512],
                                    kT[r0:r0 + 64, g, t * 128:(t + 1) * 128],
                                    qT[r0:r0 + 64, g,
                                       Hh * 1024 + c2 * 512:Hh * 1024 + (c2 + 1) * 512],
                                    start=True, stop=True,
                                )
                        pts = []
                        for hl in range(2):
                            p = ppool.tile([128, 1024], BF16, name="p")
                            nc.scalar.activation(
                                p, qks[hl], mybir.ActivationFunctionType.Exp)
                            nc.vector.tensor_mul(
                                p, p, msk[:, t, Hh * 1024:(Hh + 1) * 1024])
                            pts.append(p)
                        for hl, p in enumerate(pts):
                            h = 2 * g + hl
                            for c2 in range(2):
                                nc.tensor.matmul(
                                    avs[hl][:, c2 * 512:(c2 + 1) * 512],
                                    vs[:, t, h * 65:(h + 1) * 65],
                                    p[:, c2 * 512:(c2 + 1) * 512],
                                    start=(t == 0), stop=(t == ST - 1),
                                )
                    for hl in range(2):
                        r0 = hl * 64
                        av = avs[hl]
                        # Normalize: row 64 of av = 8*rowsum.  The reciprocal
                        # must not run on a [1, 1024] strip (iterative divide
                        # is ~8 cyc/elem on ONE lane); bounce through DRAM to
                        # reshape to [64, 16], recip there, bounce back and
                        # partition-broadcast.
                        rs = rpool.tile([65, 1024], F32, tag="rs", name="rs")
                        nc.vector.tensor_copy(rs[64:65, :], av[64:65, :])
                        rd = rdram.tile([1, 1024], F32, tag="rd", name="rd")
                        nc.sync.dma_start(rd, rs[64:65, :])
                        rsq = rpool.tile([64, 16], F32, tag="rsq", name="rsq")
                        nc.sync.dma_start(
                            rsq, rd.rearrange("one (p j) -> (one p) j", j=16))
                        rsr = rpool.tile([64, 16], F32, tag="rsr", name="rsr")
                        nc.vector.reciprocal(rsr, rsq)
                        rd2 = rdram.tile([1, 1024], F32, tag="rd2", name="rd2")
                        nc.sync.dma_start(
                            rd2.rearrange("one (p j) -> (one p) j", j=16), rsr)
                        rbc = rpool.tile([64, 1024], F32, tag="rbc", name="rbc")
                        nc.gpsimd.dma_start(
                            rbc, rd2.partition_broadcast(64))
                        nc.vector.tensor_mul(
                            att[r0:r0 + 64, g, Hh * 1024:(Hh + 1) * 1024],
                            av[0:64, :], rbc)

        # ===== Phase C: output projection =====
        with tc.tile_pool(name="wpc", bufs=1) as wpc, \
             tc.tile_pool(name="ostg", bufs=2) as ostg, \
             tc.tile_pool(name="po", bufs=4, space="PSUM") as po:
            wo_sb = wpc.tile([128, 4, D], F32R)
            nc.sync.dma_start(wo_sb, wo.rearrange("(j p) f -> p j f", p=128))
            for dm in range(ET):
                stg = ostg.tile([128, S], F32, name="stg")
                for c in range(4):
                    ps = po.tile([128, 512], F32, name="ps_o")
                    for j in range(4):
                        nc.tensor.matmul(
                            ps,
                            wo_sb[:, j, dm * 128:(dm + 1) * 128],
                            att[:, j, c * 512:(c + 1) * 512],
                            start=(j == 0), stop=(j == 3),
                        )
                    nc.scalar.copy(stg[:, c * 512:(c + 1) * 512], ps)
                nc.sync.dma_start(outT[dm * 128:(dm + 1) * 128, :], stg)


def build_nc():
    nc = bacc.Bacc("TRN2", target_bir_lowering=False, debug=False,
                   num_devices=NCORES)
    io = {}
    for name, shape, dt_, kind in [
        ("xqT", [D, S], F32R, "ExternalInput"),
        ("xkT", [D, S], F32R, "ExternalInput"),
        ("xvT", [D, S], F32R, "ExternalInput"),
        ("wq", [D, HD], F32R, "ExternalInput"),
        ("wk", [D, HD], F32R, "ExternalInput"),
        ("wv", [D, HD], F32R, "ExternalInput"),
        ("wo", [HD, D], F32R, "ExternalInput"),
        ("bqh", [128, 4], F32, "ExternalInput"),
        ("bkh", [128, 4], F32, "ExternalInput"),
        ("bvh", [HD], F32, "ExternalInput"),
        ("maskT", [S, S], BF16, "ExternalInput"),
        ("outT", [D, S], F32, "ExternalOutput"),
    ]:
        io[name] = nc.dram_tensor(name, shape, dt_, kind=kind).ap()
    with tile.TileContext(nc) as tc:
        with ExitStack() as ctx:
            _emit(ctx, tc, io)
    nc.compile()
    return nc


def make_in_maps(query, key_, value, mask, Wq, bq, Wk, bk, Wv, bv, Wo, bo):
    in_maps = []
    for c in range(NCORES):
        b, hh = c // 2, c % 2
        h0 = hh * HPC
        m01T = np.ascontiguousarray((~mask[b]).T).astype(ml_dtypes.bfloat16)
        in_maps.append({
            "xqT": np.ascontiguousarray(query[b].T),
            "xkT": np.ascontiguousarray(key_[b].T),
            "xvT": np.ascontiguousarray(value[b].T),
            "wq": np.ascontiguousarray(Wq[:, h0:h0 + HPC, :].reshape(D, HD)),
            "wk": np.ascontiguousarray(Wk[:, h0:h0 + HPC, :].reshape(D, HD)),
            "wv": np.ascontiguousarray(Wv[:, h0:h0 + HPC, :].reshape(D, HD)),
            "wo": np.ascontiguousarray(Wo[h0:h0 + HPC].reshape(HD, D)),
            "bqh": np.ascontiguousarray(
                bq[h0:h0 + HPC].reshape(4, 128).T),
            "bkh": np.ascontiguousarray(
                bk[h0:h0 + HPC].reshape(4, 128).T),
            "bvh": np.ascontiguousarray(bv[h0:h0 + HPC].reshape(HD)),
            "maskT": m01T,
        })
    return in_maps


_nc_cache = None


def kernel(query, key_, value, mask, Wq, bq, Wk, bk, Wv, bv, Wo, bo):
    global last_results, _nc_cache
    query = np.asarray(query, dtype=np.float32)
    key_ = np.asarray(key_, dtype=np.float32)
    value = np.asarray(value, dtype=np.float32)
    mask = np.asarray(mask, dtype=bool)
    Wq, bq = np.asarray(Wq, np.float32), np.asarray(bq, np.float32)
    Wk, bk = np.asarray(Wk, np.float32), np.asarray(bk, np.float32)
    Wv, bv = np.asarray(Wv, np.float32), np.asarray(bv, np.float32)
    Wo, bo = np.asarray(Wo, np.float32), np.asarray(bo, np.float32)

    if _nc_cache is None:
        _nc_cache = build_nc()
    in_maps = make_in_maps(query, key_, value, mask, Wq, bq, Wk, bk,
                           Wv, bv, Wo, bo)
    res = run_bass_kernel_spmd(_nc_cache, in_maps, core_ids=list(range(NCORES)))
    last_results = res
    out = np.empty((B, S, D), dtype=np.float32)
    for b in range(B):
        acc = res.results[2 * b]["outT"].astype(np.float32) + \
            res.results[2 * b + 1]["outT"].astype(np.float32)
        out[b] = acc.T + bo[None, :]
    return out


# revision 10
# speedup vs baseline: 1.4319x; 1.3559x over previous
"""Multi-headed attention kernel for Trainium2, SPMD across 8 NeuronCores.

Problem: B=4, S=2048, D_MODEL=1024, HEADS=16, D_HEAD=64 (fp32).

Sharding (per hint): batch across cores (4 batches x 2 cores each); within a
batch pair, heads are split 8+8 (tensor parallel). Each core computes, for its
(batch b, head half hh):
    Q^T = (Wq_s)^T X_q^T + bq   [512, 2048]   (hd-major layout, f32)
    K^T = (Wk_s)^T X_k^T + bk   [512, 2048]
    V'' = X_v Wv_s + bv         [2048, 8*65]  (bf16; per-head 65th col = 8.0)
    per head h, sq-half:  L^T = K_h Q_h^T  (f32r matmuls, PSUM f32)
                          P = exp(L^T) (bf16)  *  maskT01 (bf16, elementwise)
                          AV^T[65,sq] += V''_h^T P   (bf16 matmuls)
                          attn^T = AV^T[0:64] * (1 / AV^T[64])   (row 64 = 8*rowsum)
    out^T = Wo_s^T attn^T       [1024, 2048]  (partial over heads)
Host: out[b] = (outT_core0 + outT_core1).T + bo.

The mask is applied multiplicatively after exp: exp(l + (-1e8)) == exp(l)*0
exactly in fp32 (underflow to 0), identical to the reference's where().
No row-max subtraction is needed: logits are O(+-50), exp stays finite in f32.
"""
import numpy as np
import ml_dtypes
from contextlib import ExitStack

import concourse.bass as bass
import concourse.tile as tile
from concourse import bacc, mybir
from concourse.bass_utils import run_bass_kernel_spmd

F32 = mybir.dt.float32
F32R = mybir.dt.float32r
BF16 = mybir.dt.bfloat16

B, S, D, H, DH = 4, 2048, 1024, 16, 64
HPC = 8           # heads per core
HD = HPC * DH     # 512 head-dims per core
NCORES = 8
ET = D // 128     # 8 e-tiles (d_model contraction tiles)
ST = S // 128     # 16 s-tiles
ADD = mybir.AluOpType.add

# stash of last run results for test harness introspection
last_results = None


def _emit(ctx: ExitStack, tc: tile.TileContext, io: dict):
    nc = tc.nc
    xqT, xkT, xvT = io["xqT"], io["xkT"], io["xvT"]
    wq, wk, wv, wo = io["wq"], io["wk"], io["wv"], io["wo"]
    bqh, bkh, bvh = io["bqh"], io["bkh"], io["bvh"]
    maskT, outT = io["maskT"], io["outT"]

    const = ctx.enter_context(tc.tile_pool(name="const", bufs=1))
    bigA = ctx.enter_context(tc.tile_pool(name="bigA", bufs=1))

    # ---- constants ----
    bqh_sb = const.tile([128, 4], F32)
    nc.sync.dma_start(bqh_sb, bqh)
    bkh_sb = const.tile([128, 4], F32)
    nc.sync.dma_start(bkh_sb, bkh)
    bv_bc = const.tile([128, HD], F32)
    nc.gpsimd.dma_start(bv_bc, bvh.partition_broadcast(128))

    # ---- tensors spanning phases A..C ----
    qT = bigA.tile([128, 4, S], F32R)       # [p, hd-tile, sq]
    kT = bigA.tile([128, 4, S], F32R)
    vs = bigA.tile([128, ST, HPC * 65], BF16)  # [p, s-tile, h*65+d]; col 64 per head = 8.0

    # ===== Phase A: projections =====
    with tc.tile_pool(name="xin", bufs=9) as xin, \
         tc.tile_pool(name="wpool", bufs=2) as wpool, \
         tc.tile_pool(name="pa", bufs=4, space="PSUM") as pa:
        # --- Q^T and K^T (orientation: out[hd, sq] = W^T @ X^T) ---
        for which, (xT, w_dram, bias_sb, out_sb) in enumerate(
            [(xqT, wq, bqh_sb, qT), (xkT, wk, bkh_sb, kT)]
        ):
            w_sb = wpool.tile([128, ET, HD], F32R, tag="w", name=f"w{which}")
            nc.sync.dma_start(w_sb, w_dram.rearrange("(e p) f -> p e f", p=128))
            xts = []
            for e in range(ET):
                xt = xin.tile([128, S], F32R, tag="x", name=f"x{which}_{e}")
                nc.sync.dma_start(xt, xT[e * 128:(e + 1) * 128, :])
                xts.append(xt)
            for i in range(4):
                for c in range(4):
                    ps = pa.tile([128, 512], F32, name="ps_qk")
                    for e in range(ET):
                        nc.tensor.matmul(
                            ps,
                            w_sb[:, e, i * 128:(i + 1) * 128],
                            xts[e][:, c * 512:(c + 1) * 512],
                            start=(e == 0), stop=(e == ET - 1),
                        )
                    nc.vector.tensor_scalar(
                        out=out_sb[:, i, c * 512:(c + 1) * 512], in0=ps,
                        scalar1=bias_sb[:, i:i + 1], scalar2=None, op0=ADD,
                    )

        # --- V'' (orientation: out[s, hd] = X @ Wv) ---
        wv_sb = wpool.tile([128, ET, HD], F32R, tag="w")
        nc.sync.dma_start(wv_sb, wv.rearrange("(e p) f -> p e f", p=128))
        xvs = []
        for e in range(ET):
            xt = xin.tile([128, S], F32R, tag="x", name=f"xv_{e}")
            nc.sync.dma_start(xt, xvT[e * 128:(e + 1) * 128, :])
            xvs.append(xt)
        ones_view = vs.rearrange("p s (h dd) -> p s h dd", dd=65)[:, :, :, 64:65]
        nc.vector.memset(ones_view, 8.0)
        for s in range(ST):
            ps = pa.tile([128, 512], F32, name="ps_v")
            for e in range(ET):
                nc.tensor.matmul(
                    ps,
                    xvs[e][:, s * 128:(s + 1) * 128],
                    wv_sb[:, e, :],
                    start=(e == 0), stop=(e == ET - 1),
                )
            nc.vector.tensor_add(
                out=vs[:, s, :].rearrange("p (h dd) -> p h dd", dd=65)[:, :, 0:64],
                in0=ps.rearrange("p (h d) -> p h d", d=64),
                in1=bv_bc.rearrange("p (h d) -> p h d", d=64),
            )

    with tc.tile_pool(name="attp", bufs=1) as attp:
        att = attp.tile([128, 4, S], F32R)   # attn^T [hd, sq]

        # ===== Phase B: attention =====
        with tc.tile_pool(name="mskp", bufs=1) as mskp, \
             tc.tile_pool(name="ppool", bufs=3) as ppool, \
             tc.tile_pool(name="rpool", bufs=2) as rpool, \
             tc.tile_pool(name="rdram", bufs=2, space="DRAM") as rdram, \
             tc.tile_pool(name="pqk", bufs=3, space="PSUM") as pqk, \
             tc.tile_pool(name="pav", bufs=2, space="PSUM") as pav:
            msk = mskp.tile([128, ST, S], BF16)   # maskT01 [sk, sq]
            for t in range(ST):
                nc.sync.dma_start(msk[:, t, :], maskT[t * 128:(t + 1) * 128, :])

            for g in range(4):            # head pairs; bases 0/64 overlap on PE
                for Hq in range(4):       # sq quarters (512 cols)
                    q0 = Hq * 512
                    avs = []
                    for hl in range(2):
                        av = pav.tile([65, 512], F32, tag="av", name=f"av{hl}")
                        avs.append(av)
                    for t in range(ST):
                        # One [h0|h1] logits tile; the two K=64 matmuls sit on
                        # row strips 0/64 and stream through PE concurrently.
                        qk = pqk.tile([128, 1024], F32, name="qk")
                        for hl in range(2):
                            r0 = hl * 64
                            nc.tensor.matmul(
                                qk[:, hl * 512:(hl + 1) * 512],
                                kT[r0:r0 + 64, g, t * 128:(t + 1) * 128],
                                qT[r0:r0 + 64, g, q0:q0 + 512],
                                start=True, stop=True,
                            )
                        p = ppool.tile([128, 1024], BF16, name="p")
                        nc.scalar.activation(
                            p, qk, mybir.ActivationFunctionType.Exp)
                        # both halves are the same sq columns -> same mask
                        mslice = msk[:, t, q0:q0 + 512]
                        mrep = bass.AP(
                            tensor=mslice.tensor, offset=mslice.offset,
                            ap=[mslice.ap[0], [0, 2]] + mslice.ap[1:])
                        nc.vector.tensor_mul(
                            p.rearrange("pp (two n) -> pp two n", two=2),
                            p.rearrange("pp (two n) -> pp two n", two=2),
                            mrep)
                        for hl in range(2):
                            h = 2 * g + hl
                            nc.tensor.matmul(
                                avs[hl],
                                vs[:, t, h * 65:(h + 1) * 65],
                                p[:, hl * 512:(hl + 1) * 512],
                                start=(t == 0), stop=(t == ST - 1),
                            )
                    for hl in range(2):
                        r0 = hl * 64
                        av = avs[hl]
                        # Evacuate av quickly (frees the PSUM bank) then
                        # normalize.  Row 64 of av = 8*rowsum; reciprocal runs
                        # on a [64, 8] reshape (via DRAM bounce) since the
                        # iterative divide is ~8 cyc/elem per lane.
                        avc = rpool.tile([64, 512], F32, tag="avc", name="avc")
                        nc.vector.tensor_copy(avc, av[0:64, :])
                        rs = rpool.tile([65, 512], F32, tag="rs", name="rs")
                        nc.vector.tensor_copy(rs[64:65, :], av[64:65, :])
                        rd = rdram.tile([1, 512], F32, tag="rd", name="rd")
                        nc.sync.dma_start(rd, rs[64:65, :])
                        rsq = rpool.tile([64, 8], F32, tag="rsq", name="rsq")
                        nc.sync.dma_start(
                            rsq, rd.rearrange("one (p j) -> (one p) j", j=8))
                        rsr = rpool.tile([64, 8], F32, tag="rsr", name="rsr")
                        nc.vector.reciprocal(rsr, rsq)
                        rd2 = rdram.tile([1, 512], F32, tag="rd2", name="rd2")
                        nc.sync.dma_start(
                            rd2.rearrange("one (p j) -> (one p) j", j=8), rsr)
                        rbc = rpool.tile([64, 512], F32, tag="rbc", name="rbc")
                        nc.gpsimd.dma_start(
                            rbc, rd2.partition_broadcast(64))
                        nc.vector.tensor_mul(
                            att[r0:r0 + 64, g, q0:q0 + 512], avc, rbc)

        # ===== Phase C: output projection =====
        with tc.tile_pool(name="wpc", bufs=1) as wpc, \
             tc.tile_pool(name="ostg", bufs=2) as ostg, \
             tc.tile_pool(name="po", bufs=4, space="PSUM") as po:
            wo_sb = wpc.tile([128, 4, D], F32R)
            nc.sync.dma_start(wo_sb, wo.rearrange("(j p) f -> p j f", p=128))
            for dm in range(ET):
                stg = ostg.tile([128, S], F32, name="stg")
                for c in range(4):
                    ps = po.tile([128, 512], F32, name="ps_o")
                    for j in range(4):
                        nc.tensor.matmul(
                            ps,
                            wo_sb[:, j, dm * 128:(dm + 1) * 128],
                            att[:, j, c * 512:(c + 1) * 512],
                            start=(j == 0), stop=(j == 3),
                        )
                    nc.scalar.copy(stg[:, c * 512:(c + 1) * 512], ps)
                nc.sync.dma_start(outT[dm * 128:(dm + 1) * 128, :], stg)


def build_nc():
    nc = bacc.Bacc("TRN2", target_bir_lowering=False, debug=False,
                   num_devices=NCORES)
    io = {}
    for name, shape, dt_, kind in [
        ("xqT", [D, S], F32R, "ExternalInput"),
        ("xkT", [D, S], F32R, "ExternalInput"),
        ("xvT", [D, S], F32R, "ExternalInput"),
        ("wq", [D, HD], F32R, "ExternalInput"),
        ("wk", [D, HD], F32R, "ExternalInput"),
        ("wv", [D, HD], F32R, "ExternalInput"),
        ("wo", [HD, D], F32R, "ExternalInput"),
        ("bqh", [128, 4], F32, "ExternalInput"),
        ("bkh", [128, 4], F32, "ExternalInput"),
        ("bvh", [HD], F32, "ExternalInput"),
        ("maskT", [S, S], BF16, "ExternalInput"),
        ("outT", [D, S], F32, "ExternalOutput"),
    ]:
        io[name] = nc.dram_tensor(name, shape, dt_, kind=kind).ap()
    with tile.TileContext(nc) as tc:
        with ExitStack() as ctx:
            _emit(ctx, tc, io)
    nc.compile()
    return nc


def make_in_maps(query, key_, value, mask, Wq, bq, Wk, bk, Wv, bv, Wo, bo):
    in_maps = []
    for c in range(NCORES):
        b, hh = c // 2, c % 2
        h0 = hh * HPC
        m01T = np.ascontiguousarray((~mask[b]).T).astype(ml_dtypes.bfloat16)
        in_maps.append({
            "xqT": np.ascontiguousarray(query[b].T),
            "xkT": np.ascontiguousarray(key_[b].T),
            "xvT": np.ascontiguousarray(value[b].T),
            "wq": np.ascontiguousarray(Wq[:, h0:h0 + HPC, :].reshape(D, HD)),
            "wk": np.ascontiguousarray(Wk[:, h0:h0 + HPC, :].reshape(D, HD)),
            "wv": np.ascontiguousarray(Wv[:, h0:h0 + HPC, :].reshape(D, HD)),
            "wo": np.ascontiguousarray(Wo[h0:h0 + HPC].reshape(HD, D)),
            "bqh": np.ascontiguousarray(
                bq[h0:h0 + HPC].reshape(4, 128).T),
            "bkh": np.ascontiguousarray(
                bk[h0:h0 + HPC].reshape(4, 128).T),
            "bvh": np.ascontiguousarray(bv[h0:h0 + HPC].reshape(HD)),
            "maskT": m01T,
        })
    return in_maps


_nc_cache = None


def kernel(query, key_, value, mask, Wq, bq, Wk, bk, Wv, bv, Wo, bo):
    global last_results, _nc_cache
    query = np.asarray(query, dtype=np.float32)
    key_ = np.asarray(key_, dtype=np.float32)
    value = np.asarray(value, dtype=np.float32)
    mask = np.asarray(mask, dtype=bool)
    Wq, bq = np.asarray(Wq, np.float32), np.asarray(bq, np.float32)
    Wk, bk = np.asarray(Wk, np.float32), np.asarray(bk, np.float32)
    Wv, bv = np.asarray(Wv, np.float32), np.asarray(bv, np.float32)
    Wo, bo = np.asarray(Wo, np.float32), np.asarray(bo, np.float32)

    if _nc_cache is None:
        _nc_cache = build_nc()
    in_maps = make_in_maps(query, key_, value, mask, Wq, bq, Wk, bk,
                           Wv, bv, Wo, bo)
    res = run_bass_kernel_spmd(_nc_cache, in_maps, core_ids=list(range(NCORES)))
    last_results = res
    out = np.empty((B, S, D), dtype=np.float32)
    for b in range(B):
        acc = res.results[2 * b]["outT"].astype(np.float32) + \
            res.results[2 * b + 1]["outT"].astype(np.float32)
        out[b] = acc.T + bo[None, :]
    return out


# revision 14
# speedup vs baseline: 1.5808x; 1.1039x over previous
"""Multi-headed attention kernel for Trainium2, SPMD across 8 NeuronCores.

Problem: B=4, S=2048, D_MODEL=1024, HEADS=16, D_HEAD=64 (fp32).

Sharding (per hint): batch across cores (4 batches x 2 cores each); within a
batch pair, heads are split 8+8 (tensor parallel). Each core computes, for its
(batch b, head half hh):
    Q^T = (Wq_s)^T X_q^T + bq   [512, 2048]   (hd-major layout, f32)
    K^T = (Wk_s)^T X_k^T + bk   [512, 2048]
    V'' = X_v Wv_s + bv         [2048, 8*65]  (bf16; per-head 65th col = 8.0)
    per head h, sq-half:  L^T = K_h Q_h^T  (f32r matmuls, PSUM f32)
                          P = exp(L^T) (bf16)  *  maskT01 (bf16, elementwise)
                          AV^T[65,sq] += V''_h^T P   (bf16 matmuls)
                          attn^T = AV^T[0:64] * (1 / AV^T[64])   (row 64 = 8*rowsum)
    out^T = Wo_s^T attn^T       [1024, 2048]  (partial over heads)
Host: out[b] = (outT_core0 + outT_core1).T + bo.

The mask is applied multiplicatively after exp: exp(l + (-1e8)) == exp(l)*0
exactly in fp32 (underflow to 0), identical to the reference's where().
No row-max subtraction is needed: logits are O(+-50), exp stays finite in f32.
"""
import numpy as np
import ml_dtypes
from contextlib import ExitStack

import concourse.bass as bass
import concourse.tile as tile
from concourse import bacc, mybir
from concourse.bass_utils import run_bass_kernel_spmd

F32 = mybir.dt.float32
F32R = mybir.dt.float32r
BF16 = mybir.dt.bfloat16

B, S, D, H, DH = 4, 2048, 1024, 16, 64
HPC = 8           # heads per core
HD = HPC * DH     # 512 head-dims per core
NCORES = 8
ET = D // 128     # 8 e-tiles (d_model contraction tiles)
ST = S // 128     # 16 s-tiles
ADD = mybir.AluOpType.add

# stash of last run results for test harness introspection
last_results = None


def _emit(ctx: ExitStack, tc: tile.TileContext, io: dict):
    nc = tc.nc
    xqT, xkT, xvT = io["xqT"], io["xkT"], io["xvT"]
    wq, wk, wv, wo = io["wq"], io["wk"], io["wv"], io["wo"]
    bqh, bkh, bvh = io["bqh"], io["bkh"], io["bvh"]
    maskT, outT = io["maskT"], io["outT"]

    const = ctx.enter_context(tc.tile_pool(name="const", bufs=1))
    bigA = ctx.enter_context(tc.tile_pool(name="bigA", bufs=1))

    # ---- constants ----
    bqh_sb = const.tile([128, 4], F32)
    nc.sync.dma_start(bqh_sb, bqh)
    bkh_sb = const.tile([128, 4], F32)
    nc.sync.dma_start(bkh_sb, bkh)
    bv_bc = const.tile([128, HD], F32)
    nc.gpsimd.dma_start(bv_bc, bvh.partition_broadcast(128))

    # ---- tensors spanning phases A..C ----
    qT = bigA.tile([128, 4, S], F32R)       # [p, hd-tile, sq]
    kT = bigA.tile([128, 4, S], F32R)
    vs = bigA.tile([128, ST, HPC * 65], BF16)  # [p, s-tile, h*65+d]; col 64 per head = 8.0

    # ---- HAM warm-up: ~5us of bf16 matmuls raise the PE clock gate to
    # 8/8 (2.4 GHz).  f32r matmuls preserve but do not establish the warm
    # state, so prime before the f32r-heavy phases. ----
    with tc.tile_pool(name="warm", bufs=1) as warm, \
         tc.tile_pool(name="pwarm", bufs=2, space="PSUM") as pwarm:
        wz = warm.tile([128, 1024], BF16)
        nc.vector.memset(wz, 0.0)
        for i in range(26):
            pw = pwarm.tile([128, 512], F32, name="pw", tag="pw")
            nc.tensor.matmul(pw, wz[:, 0:128], wz[:, 0:512],
                             start=True, stop=True)

    # ===== Phase A: projections =====
    with tc.tile_pool(name="xin", bufs=36) as xin, \
         tc.tile_pool(name="wpool", bufs=2) as wpool, \
         tc.tile_pool(name="pa", bufs=4, space="PSUM") as pa:
        # --- Q^T and K^T (orientation: out[hd, sq] = W^T @ X^T) ---
        for which, (xT, w_dram, bias_sb, out_sb) in enumerate(
            [(xqT, wq, bqh_sb, qT), (xkT, wk, bkh_sb, kT)]
        ):
            w_sb = wpool.tile([128, ET, HD], F32R, tag="w", name=f"w{which}")
            nc.sync.dma_start(w_sb, w_dram.rearrange("(e p) f -> p e f", p=128))
            # column-chunked loads: the first matmul group only waits on the
            # first 8 chunks (2 MB), not the whole 8 MB input
            xts = {}
            for c in range(4):
                for e in range(ET):
                    xt = xin.tile([128, 512], F32R, tag="x",
                                  name=f"x{which}_{e}_{c}")
                    nc.sync.dma_start(
                        xt, xT[e * 128:(e + 1) * 128, c * 512:(c + 1) * 512])
                    xts[(e, c)] = xt
            for i in range(4):
                for c in range(4):
                    ps = pa.tile([128, 512], F32, name="ps_qk")
                    for e in range(ET):
                        nc.tensor.matmul(
                            ps,
                            w_sb[:, e, i * 128:(i + 1) * 128],
                            xts[(e, c)],
                            start=(e == 0), stop=(e == ET - 1),
                        )
                    nc.vector.tensor_scalar(
                        out=out_sb[:, i, c * 512:(c + 1) * 512], in0=ps,
                        scalar1=bias_sb[:, i:i + 1], scalar2=None, op0=ADD,
                    )

        # --- V'' (orientation: out[s, hd] = X @ Wv) ---
        wv_sb = wpool.tile([128, ET, HD], F32R, tag="w")
        nc.sync.dma_start(wv_sb, wv.rearrange("(e p) f -> p e f", p=128))
        xvs = {}
        for c in range(4):
            for e in range(ET):
                xt = xin.tile([128, 512], F32R, tag="x", name=f"xv_{e}_{c}")
                nc.sync.dma_start(
                    xt, xvT[e * 128:(e + 1) * 128, c * 512:(c + 1) * 512])
                xvs[(e, c)] = xt
        ones_view = vs.rearrange("p s (h dd) -> p s h dd", dd=65)[:, :, :, 64:65]
        nc.vector.memset(ones_view, 8.0)
        for s in range(ST):
            ps = pa.tile([128, 512], F32, name="ps_v")
            for e in range(ET):
                nc.tensor.matmul(
                    ps,
                    xvs[(e, s // 4)][:, (s % 4) * 128:(s % 4 + 1) * 128],
                    wv_sb[:, e, :],
                    start=(e == 0), stop=(e == ET - 1),
                )
            nc.vector.tensor_add(
                out=vs[:, s, :].rearrange("p (h dd) -> p h dd", dd=65)[:, :, 0:64],
                in0=ps.rearrange("p (h d) -> p h d", d=64),
                in1=bv_bc.rearrange("p (h d) -> p h d", d=64),
            )

    with tc.tile_pool(name="attp", bufs=1) as attp:
        att = attp.tile([128, 4, S], F32R)   # attn^T [hd, sq]

        # ===== Phase B: attention =====
        with tc.tile_pool(name="mskp", bufs=1) as mskp, \
             tc.tile_pool(name="ppool", bufs=3) as ppool, \
             tc.tile_pool(name="rpool", bufs=2) as rpool, \
             tc.tile_pool(name="rdram", bufs=2, space="DRAM") as rdram, \
             tc.tile_pool(name="pqk", bufs=2, space="PSUM") as pqk, \
             tc.tile_pool(name="pav", bufs=4, space="PSUM") as pav:
            msk = mskp.tile([128, ST, S], BF16)   # maskT01 [sk, sq]
            for t in range(ST):
                nc.sync.dma_start(msk[:, t, :], maskT[t * 128:(t + 1) * 128, :])

            for g in range(4):            # head pairs; bases 0/64 overlap on PE
                for Hq in range(4):       # sq quarters (512 cols)
                    q0 = Hq * 512
                    avs = []
                    for hl in range(2):
                        av = pav.tile([65, 512], F32, tag="av", name=f"av{hl}")
                        avs.append(av)
                    for t in range(ST):
                        # One [h0|h1] logits tile; the two K=64 matmuls sit on
                        # row strips 0/64 and stream through PE concurrently.
                        qk = pqk.tile([128, 1024], F32, name="qk")
                        for hl in range(2):
                            r0 = hl * 64
                            nc.tensor.matmul(
                                qk[:, hl * 512:(hl + 1) * 512],
                                kT[r0:r0 + 64, g, t * 128:(t + 1) * 128],
                                qT[r0:r0 + 64, g, q0:q0 + 512],
                                start=True, stop=True,
                            )
                        p = ppool.tile([128, 1024], BF16, name="p")
                        nc.scalar.activation(
                            p, qk, mybir.ActivationFunctionType.Exp)
                        # both halves are the same sq columns -> same mask
                        mslice = msk[:, t, q0:q0 + 512]
                        mrep = bass.AP(
                            tensor=mslice.tensor, offset=mslice.offset,
                            ap=[mslice.ap[0], [0, 2]] + mslice.ap[1:])
                        nc.vector.tensor_mul(
                            p.rearrange("pp (two n) -> pp two n", two=2),
                            p.rearrange("pp (two n) -> pp two n", two=2),
                            mrep)
                        for hl in range(2):
                            h = 2 * g + hl
                            nc.tensor.matmul(
                                avs[hl],
                                vs[:, t, h * 65:(h + 1) * 65],
                                p[:, hl * 512:(hl + 1) * 512],
                                start=(t == 0), stop=(t == ST - 1),
                            )
                    for hl in range(2):
                        r0 = hl * 64
                        av = avs[hl]
                        # Evacuate av quickly (frees the PSUM bank) then
                        # normalize.  Row 64 of av = 8*rowsum; reciprocal runs
                        # on a [64, 8] reshape (via DRAM bounce) since the
                        # iterative divide is ~8 cyc/elem per lane.
                        avc = rpool.tile([64, 512], F32, tag="avc", name="avc")
                        nc.vector.tensor_copy(avc, av[0:64, :])
                        rs = rpool.tile([65, 512], F32, tag="rs", name="rs")
                        nc.vector.tensor_copy(rs[64:65, :], av[64:65, :])
                        rd = rdram.tile([1, 512], F32, tag="rd", name="rd")
                        nc.sync.dma_start(rd, rs[64:65, :])
                        rsq = rpool.tile([64, 8], F32, tag="rsq", name="rsq")
                        nc.sync.dma_start(
                            rsq, rd.rearrange("one (p j) -> (one p) j", j=8))
                        rsr = rpool.tile([64, 8], F32, tag="rsr", name="rsr")
                        nc.vector.reciprocal(rsr, rsq)
                        rd2 = rdram.tile([1, 512], F32, tag="rd2", name="rd2")
                        nc.sync.dma_start(
                            rd2.rearrange("one (p j) -> (one p) j", j=8), rsr)
                        rbc = rpool.tile([64, 512], F32, tag="rbc", name="rbc")
                        nc.gpsimd.dma_start(
                            rbc, rd2.partition_broadcast(64))
                        nc.vector.tensor_mul(
                            att[r0:r0 + 64, g, q0:q0 + 512], avc, rbc)

        # ===== Phase C: output projection =====
        with tc.tile_pool(name="wpc", bufs=1) as wpc, \
             tc.tile_pool(name="ostg", bufs=2) as ostg, \
             tc.tile_pool(name="po", bufs=4, space="PSUM") as po:
            wo_sb = wpc.tile([128, 4, D], F32R)
            nc.sync.dma_start(wo_sb, wo.rearrange("(j p) f -> p j f", p=128))
            for dm in range(ET):
                stg = ostg.tile([128, S], F32, name="stg")
                for c in range(4):
                    ps = po.tile([128, 512], F32, name="ps_o")
                    for j in range(4):
                        nc.tensor.matmul(
                            ps,
                            wo_sb[:, j, dm * 128:(dm + 1) * 128],
                            att[:, j, c * 512:(c + 1) * 512],
                            start=(j == 0), stop=(j == 3),
                        )
                    nc.scalar.copy(stg[:, c * 512:(c + 1) * 512], ps)
                nc.sync.dma_start(outT[dm * 128:(dm + 1) * 128, :], stg)


def build_nc():
    nc = bacc.Bacc("TRN2", target_bir_lowering=False, debug=False,
                   num_devices=NCORES)
    io = {}
    for name, shape, dt_, kind in [
        ("xqT", [D, S], F32R, "ExternalInput"),
        ("xkT", [D, S], F32R, "ExternalInput"),
        ("xvT", [D, S], F32R, "ExternalInput"),
        ("wq", [D, HD], F32R, "ExternalInput"),
        ("wk", [D, HD], F32R, "ExternalInput"),
        ("wv", [D, HD], F32R, "ExternalInput"),
        ("wo", [HD, D], F32R, "ExternalInput"),
        ("bqh", [128, 4], F32, "ExternalInput"),
        ("bkh", [128, 4], F32, "ExternalInput"),
        ("bvh", [HD], F32, "ExternalInput"),
        ("maskT", [S, S], BF16, "ExternalInput"),
        ("outT", [D, S], F32, "ExternalOutput"),
    ]:
        io[name] = nc.dram_tensor(name, shape, dt_, kind=kind).ap()
    with tile.TileContext(nc) as tc:
        with ExitStack() as ctx:
            _emit(ctx, tc, io)
    nc.compile()
    return nc


def make_in_maps(query, key_, value, mask, Wq, bq, Wk, bk, Wv, bv, Wo, bo):
    in_maps = []
    for c in range(NCORES):
        b, hh = c // 2, c % 2
        h0 = hh * HPC
        m01T = np.ascontiguousarray((~mask[b]).T).astype(ml_dtypes.bfloat16)
        in_maps.append({
            "xqT": np.ascontiguousarray(query[b].T),
            "xkT": np.ascontiguousarray(key_[b].T),
            "xvT": np.ascontiguousarray(value[b].T),
            "wq": np.ascontiguousarray(Wq[:, h0:h0 + HPC, :].reshape(D, HD)),
            "wk": np.ascontiguousarray(Wk[:, h0:h0 + HPC, :].reshape(D, HD)),
            "wv": np.ascontiguousarray(Wv[:, h0:h0 + HPC, :].reshape(D, HD)),
            "wo": np.ascontiguousarray(Wo[h0:h0 + HPC].reshape(HD, D)),
            "bqh": np.ascontiguousarray(
                bq[h0:h0 + HPC].reshape(4, 128).T),
            "bkh": np.ascontiguousarray(
                bk[h0:h0 + HPC].reshape(4, 128).T),
            "bvh": np.ascontiguousarray(bv[h0:h0 + HPC].reshape(HD)),
            "maskT": m01T,
        })
    return in_maps


_nc_cache = None


def kernel(query, key_, value, mask, Wq, bq, Wk, bk, Wv, bv, Wo, bo):
    global last_results, _nc_cache
    query = np.asarray(query, dtype=np.float32)
    key_ = np.asarray(key_, dtype=np.float32)
    value = np.asarray(value, dtype=np.float32)
    mask = np.asarray(mask, dtype=bool)
    Wq, bq = np.asarray(Wq, np.float32), np.asarray(bq, np.float32)
    Wk, bk = np.asarray(Wk, np.float32), np.asarray(bk, np.float32)
    Wv, bv = np.asarray(Wv, np.float32), np.asarray(bv, np.float32)
    Wo, bo = np.asarray(Wo, np.float32), np.asarray(bo, np.float32)

    if _nc_cache is None:
        _nc_cache = build_nc()
    in_maps = make_in_maps(query, key_, value, mask, Wq, bq, Wk, bk,
                           Wv, bv, Wo, bo)
    res = run_bass_kernel_spmd(_nc_cache, in_maps, core_ids=list(range(NCORES)))
    last_results = res
    out = np.empty((B, S, D), dtype=np.float32)
    for b in range(B):
        acc = res.results[2 * b]["outT"].astype(np.float32) + \
            res.results[2 * b + 1]["outT"].astype(np.float32)
        out[b] = acc.T + bo[None, :]
    return out
